# revision 10
# baseline (speedup 1.0000x reference)
"""Chunkwise causal attention (full causal MHA + QKV/out projections) on 8 trn2 cores.

Sharding: data-parallel over batch (B=2) x tensor-parallel over heads (16 -> 4 per
core). Each core computes, for its (batch, 4-head group): qkv projection (bf16
matmuls, f32 accumulation), causal attention (bf16 QK, fp16 PV with a fused
ones-column providing softmax denominators), and its partial out-projection
(fp16). Host sums the 4 fp16 partials per batch and adds the output bias.

Self-contained: hardcodes all shapes from the problem spec.
"""

import numpy as np
import ml_dtypes

import concourse.bass as bass
import concourse.mybir as mybir
import concourse.tile as tile
from concourse import bacc
from concourse.bass_utils import run_bass_kernel_spmd
from concourse.masks import make_identity

# Problem shapes
B, S, D = 2, 2048, 2048
H, Dh = 16, 128
HC = 4                      # heads per core
P = 128
SPLIT = 4
SQ = S // SPLIT             # 512: S-chunk processed per outer phase
N_DC = D // P               # 16 contraction chunks for projections
N_SC = S // P               # 16 S chunks
SCALE = 1.0 / float(np.sqrt(Dh))
VW = 132                    # v_aug row width (129 used: 128 dh + 1 ones col)

f32 = mybir.dt.float32
bf16 = mybir.dt.bfloat16
f16 = mybir.dt.float16

np_bf16 = ml_dtypes.bfloat16

_COMPILED = {}

# Tunable pool sizes (PSUM pools must keep total banks <= 8)
CFG = dict(WBUFS=1, EXPBUFS=19, QKPSB=2, TRB=1, QK2B=2, PVB=1, SPB=4, OSB=3,
           OPSB=2)


def build_program():
    nc = bacc.Bacc("TRN2", target_bir_lowering=False, debug=False)

    xT = nc.dram_tensor("xT", (SPLIT, P, N_DC, SQ), bf16, kind="ExternalInput")
    wqkv = nc.dram_tensor("wqkv", (P, 3, HC, N_DC, Dh), bf16,
                          kind="ExternalInput")
    wout = nc.dram_tensor("wout", (P, HC, D), f16, kind="ExternalInput")
    aux = nc.dram_tensor("aux", (P, 8 + HC * Dh), f32, kind="ExternalInput")
    masks = nc.dram_tensor("masks", (P, 4, 512), f16, kind="ExternalInput")
    outp = nc.dram_tensor("outp", (S, D), f16, kind="ExternalOutput")

    xT_ap, wqkv_ap, wout_ap = xT.ap(), wqkv.ap(), wout.ap()
    masks_ap, outp_ap = masks.ap(), outp.ap()

    with tile.TileContext(nc, trace_sim=CFG.get('TRACE', False)) as tc:
        with tc.tile_pool(name="const", bufs=1) as const, \
             tc.tile_pool(name="persist", bufs=1) as persist:

            # The first proj chain needs w(h0,q) + xT(sf0): issue those
            # first on the sync (SP HWDGE) ring — xT0 in 4 chunks so the
            # chain starts mid-transfer — then the remaining weights and
            # wout behind them on the same ring. aux/masks ride the scalar
            # (ACT HWDGE) ring so nothing shares bandwidth with the
            # critical path at t=0.
            w_sb = persist.tile([P, 3, HC, N_DC, Dh], bf16, tag="wqkv")
            nc.scalar.dma_start(w_sb[:, 0, 0], wqkv_ap[:, 0, 0])
            xT0_sb = const.tile([P, N_DC, SQ], bf16, tag="xT0")
            for c in range(4):
                nc.sync.dma_start(xT0_sb[:, 4 * c:4 * (c + 1)],
                                  xT_ap[0, :, 4 * c:4 * (c + 1)])
            for h in range(HC):
                for part in range(3):
                    if (h, part) == (0, 0):
                        continue
                    nc.sync.dma_start(w_sb[:, part, h], wqkv_ap[:, part, h])

            ident = const.tile([P, P], f32, tag="ident")
            make_identity(nc, ident)
            ident16 = const.tile([P, P], f16, tag="ident16")
            nc.scalar.copy(ident16, ident)
            aux_sb = const.tile([P, 8 + HC * Dh], f32, tag="aux")
            nc.scalar.dma_start(aux_sb[:], aux.ap())
            masks_sb = const.tile([P, 4, 512], f16, tag="masks")
            nc.scalar.dma_start(masks_sb[:], masks_ap)
            bq_sb = aux_sb[:, 0:HC]
            bk_sb = aux_sb[:, HC:2 * HC]
            bvb_sb = aux_sb[:, 8:8 + HC * Dh]

            wo_sb = persist.tile([P, HC, D], f16, tag="wo")
            for h in range(HC):
                nc.sync.dma_start(wo_sb[:, h], wout_ap[:, h])

            kT_sb = persist.tile([P, HC, S], bf16, tag="kT")
            vaug_sb = persist.tile([P, HC, N_SC, VW], f16, tag="vaug")
            # ones column for softmax denominators
            nc.vector.memset(vaug_sb[:, :, :, Dh:Dh + 1], 1.0)

            with tc.tile_pool(name="abpool", bufs=2) as abpool, \
                 tc.tile_pool(name="exppool", bufs=CFG["EXPBUFS"]) as exppool, \
                 tc.tile_pool(name="spool", bufs=CFG["SPB"]) as spool, \
                 tc.tile_pool(name="qkps", bufs=CFG["QKPSB"], space="PSUM") as qkps, \
                 tc.tile_pool(name="trps", bufs=CFG["TRB"], space="PSUM") as trps, \
                 tc.tile_pool(name="qk2ps", bufs=CFG["QK2B"], space="PSUM") as qk2ps, \
                 tc.tile_pool(name="pvps", bufs=CFG["PVB"], space="PSUM") as pvps, \
                 tc.tile_pool(name="osb", bufs=CFG["OSB"]) as osb, \
                 tc.tile_pool(name="ops", bufs=CFG["OPSB"], space="PSUM") as ops:

                for sf in range(SPLIT):
                    # ---- Phase A: projections for this S-chunk ----
                    if sf == 0:
                        xT_sb = xT0_sb
                    else:
                        xT_sb = abpool.tile([P, N_DC, SQ], bf16, tag="xT")
                        nc.sync.dma_start(xT_sb[:], xT_ap[sf])
                    qT_sb = abpool.tile([P, HC, SQ], bf16, tag="qT")
                    vT_sb = abpool.tile([P, HC, SQ], f16, tag="vT")

                    for h in range(HC):
                        for part in range(3):
                            ps = qkps.tile([P, SQ], f32, tag="projps")
                            for dc in range(N_DC):
                                nc.tensor.matmul(
                                    ps,
                                    w_sb[:, part, h, dc],
                                    xT_sb[:, dc],
                                    start=(dc == 0), stop=(dc == N_DC - 1))
                            if part == 0:
                                nc.vector.tensor_scalar_add(
                                    qT_sb[:, h], ps, bq_sb[:, h:h + 1])
                            elif part == 1:
                                nc.vector.tensor_scalar_add(
                                    kT_sb[:, h, sf * SQ:(sf + 1) * SQ],
                                    ps, bk_sb[:, h:h + 1])
                            else:
                                nc.vector.tensor_copy(vT_sb[:, h], ps)

                    # v: transpose to natural layout, add bias, cast to fp16
                    for h in range(HC):
                        for scl in range(SQ // P):
                            sc = sf * (SQ // P) + scl
                            tp = trps.tile([P, P], f16, tag="tr")
                            nc.tensor.transpose(
                                tp, vT_sb[:, h, scl * P:(scl + 1) * P], ident16)
                            nc.vector.tensor_add(
                                vaug_sb[:, h, sc, 0:Dh], tp,
                                bvb_sb[:, h * Dh:(h + 1) * Dh])

                    # ---- Phase B: attention for q rows in this S-chunk ----
                    attnT_sb = abpool.tile([P, HC, SQ], f16, tag="attnT")
                    for h in range(HC):
                        q0 = sf * SQ
                        nk = (q0 + SQ) // P
                        exps = []
                        for kc in range(nk):
                            j = kc - (nk - 4)
                            # columns < j*128 of this chunk are fully masked
                            off = 0 if j < 1 else j * P
                            w = SQ - off
                            qk = qk2ps.tile([P, SQ], f32, tag="qk")
                            nc.tensor.matmul(
                                qk[:, 0:w],
                                kT_sb[:, h, kc * P:(kc + 1) * P],
                                qT_sb[:, h, off:SQ],
                                start=True, stop=True)
                            ex = exppool.tile([P, SQ], f16, tag="exp")
                            nc.scalar.activation(
                                ex[:, off:SQ], qk[:, 0:w],
                                mybir.ActivationFunctionType.Exp,
                                scale=SCALE)
                            if j >= 0:
                                nc.vector.tensor_mul(
                                    ex[:, off:SQ], ex[:, off:SQ],
                                    masks_sb[:, j, off:SQ])
                            exps.append(ex)
                        for sub in range(4):
                            nkq = sf * 4 + sub + 1
                            ps = pvps.tile([P, VW], f32, tag="pv")
                            for kc in range(nkq):
                                nc.tensor.matmul(
                                    ps[:, 0:Dh + 1],
                                    exps[kc][:, sub * P:(sub + 1) * P],
                                    vaug_sb[:, h, kc, 0:Dh + 1],
                                    start=(kc == 0),
                                    stop=(kc == nkq - 1))
                            rc = spool.tile([P, 1], f32, tag="rc")
                            nc.vector.reciprocal(rc, ps[:, Dh:Dh + 1])
                            at = spool.tile([P, P], f16, tag="at")
                            nc.vector.tensor_mul(
                                at, ps[:, 0:Dh], rc.to_broadcast((P, P)))
                            tp = trps.tile([P, P], f16, tag="tr")
                            nc.tensor.transpose(tp, at, ident16)
                            nc.vector.tensor_copy(
                                attnT_sb[:, h, sub * P:(sub + 1) * P], tp)

                    # ---- Phase C (this S-chunk): partial out projection ----
                    for ssl in range(SQ // P):
                        ss = sf * (SQ // P) + ssl
                        ot = osb.tile([P, D], f16, tag="ot")
                        for n in range(D // 512):
                            ps_o = ops.tile([P, 512], f32, tag="ops")
                            for hc in range(HC):
                                nc.tensor.matmul(
                                    ps_o,
                                    attnT_sb[:, hc, ssl * P:(ssl + 1) * P],
                                    wo_sb[:, hc, n * 512:(n + 1) * 512],
                                    start=(hc == 0),
                                    stop=(hc == HC - 1))
                            nc.vector.tensor_copy(ot[:, n * 512:(n + 1) * 512],
                                                  ps_o)
                        nc.sync.dma_start(outp_ap[ss * P:(ss + 1) * P], ot)

    nc.compile()
    return nc


def shard_inputs(x, Wqkv, bqkv, Wout):
    """Build the 8 per-core input maps."""
    mask = np.zeros((P, 4, 512), np.float16)
    kk = np.arange(P)[:, None]
    qq = np.arange(512)[None, :]
    for j in range(4):
        mask[:, j] = (qq >= kk + P * j).astype(np.float16)

    # xT packed per batch: [sf, p, dc, q] so each sf slice is one
    # contiguous-per-partition DMA
    xTs = [np.ascontiguousarray(
        x[b].T.reshape(N_DC, P, SPLIT, SQ).transpose(2, 1, 0, 3))
        .astype(np_bf16) for b in range(B)]
    # Wqkv [D, 3*H*Dh] -> [p, part, h, dc, dh]; per-head-group slices are
    # contiguous copies in bf16
    W_all = Wqkv.reshape(N_DC, P, 3, H, Dh).transpose(1, 2, 3, 0, 4)
    wout_f16 = Wout.astype(np.float16)
    per_hg = {}
    for hg in range(4):
        h0 = hg * HC
        c0 = h0 * Dh
        cw = HC * Dh
        aux = np.zeros((P, 8 + cw), np.float32)
        aux[:, 0:HC] = bqkv[c0:c0 + cw].reshape(HC, P).T
        aux[:, HC:2 * HC] = (bqkv[H * Dh + c0:H * Dh + c0 + cw]
                             .reshape(HC, P).T)
        aux[:, 8:8 + cw] = bqkv[2 * H * Dh + c0:2 * H * Dh + c0 + cw][None, :]
        per_hg[hg] = dict(
            wqkv=np.ascontiguousarray(W_all[:, :, h0:h0 + HC]).astype(np_bf16),
            wout=np.ascontiguousarray(
                wout_f16[c0:c0 + cw].reshape(HC, P, D).transpose(1, 0, 2)),
            aux=aux,
        )
    in_maps = []
    for c in range(8):
        b, hg = divmod(c, 4)
        g = per_hg[hg]
        in_maps.append({
            "xT": xTs[b], "wqkv": g["wqkv"], "wout": g["wout"],
            "aux": g["aux"], "masks": mask,
        })
    return in_maps


def kernel(x, Wqkv, bqkv, Wout, bout):
    x = np.asarray(x, dtype=np.float32)
    Wqkv = np.asarray(Wqkv, dtype=np.float32)
    bqkv = np.asarray(bqkv, dtype=np.float32)
    Wout = np.asarray(Wout, dtype=np.float32)
    bout = np.asarray(bout, dtype=np.float32)

    if "nc" not in _COMPILED:
        _COMPILED["nc"] = build_program()
    nc = _COMPILED["nc"]

    in_maps = shard_inputs(x, Wqkv, bqkv, Wout)
    res = run_bass_kernel_spmd(nc, in_maps, core_ids=list(range(8)))

    out = np.empty((B, S, D), np.float32)
    for b in range(B):
        np.copyto(out[b], res.results[4 * b]["outp"].astype(np.float32))
        for c in range(4 * b + 1, 4 * b + 4):
            np.add(out[b], res.results[c]["outp"].astype(np.float32),
                   out=out[b])
        out[b] += bout[None, :]
    return out


# revision 11
# speedup vs baseline: 1.0841x; 1.0841x over previous
"""Chunkwise causal attention (full causal MHA + QKV/out projections) on 8 trn2 cores.

Sharding: data-parallel over batch (B=2) x tensor-parallel over heads (16 -> 4 per
core). Each core computes, for its (batch, 4-head group): qkv projection (bf16
matmuls, f32 accumulation), causal attention (bf16 QK, fp16 PV with a fused
ones-column providing softmax denominators), and its partial out-projection
(fp16). Host sums the 4 fp16 partials per batch and adds the output bias.

Self-contained: hardcodes all shapes from the problem spec.
"""

import numpy as np
import ml_dtypes

import concourse.bass as bass
import concourse.mybir as mybir
import concourse.tile as tile
from concourse import bacc
from concourse.bass_utils import run_bass_kernel_spmd
from concourse.masks import make_identity

# Problem shapes
B, S, D = 2, 2048, 2048
H, Dh = 16, 128
HC = 4                      # heads per core
P = 128
SPLIT = 4
SQ = S // SPLIT             # 512: S-chunk processed per outer phase
N_DC = D // P               # 16 contraction chunks for projections
N_SC = S // P               # 16 S chunks
SCALE = 1.0 / float(np.sqrt(Dh))
VW = 132                    # v_aug row width (129 used: 128 dh + 1 ones col)

f32 = mybir.dt.float32
bf16 = mybir.dt.bfloat16
f16 = mybir.dt.float16

np_bf16 = ml_dtypes.bfloat16

_COMPILED = {}

# Tunable pool sizes (PSUM pools must keep total banks <= 8)
CFG = dict(WBUFS=1, EXPBUFS=19, QKPSB=2, TRB=1, QK2B=2, PVB=1, SPB=4, OSB=3,
           OPSB=2)


def build_program():
    nc = bacc.Bacc("TRN2", target_bir_lowering=False, debug=False)

    xT = nc.dram_tensor("xT", (SPLIT, P, N_DC, SQ), bf16, kind="ExternalInput")
    wqkv = nc.dram_tensor("wqkv", (P, 3, HC, N_DC, Dh), bf16,
                          kind="ExternalInput")
    wout = nc.dram_tensor("wout", (P, HC, D), f16, kind="ExternalInput")
    aux = nc.dram_tensor("aux", (P, 8 + HC * Dh), f32, kind="ExternalInput")
    masks = nc.dram_tensor("masks", (P, 4, 512), f16, kind="ExternalInput")
    outp = nc.dram_tensor("outp", (S, D), f16, kind="ExternalOutput")

    xT_ap, wqkv_ap, wout_ap = xT.ap(), wqkv.ap(), wout.ap()
    masks_ap, outp_ap = masks.ap(), outp.ap()

    with tile.TileContext(nc, trace_sim=CFG.get('TRACE', False)) as tc:
        with tc.tile_pool(name="const", bufs=1) as const, \
             tc.tile_pool(name="persist", bufs=1) as persist:

            # The first proj chain needs w(h0,q) + xT(sf0): issue those
            # first on the sync (SP HWDGE) ring — xT0 in 4 chunks so the
            # chain starts mid-transfer — then the remaining weights and
            # wout behind them on the same ring. aux/masks ride the scalar
            # (ACT HWDGE) ring so nothing shares bandwidth with the
            # critical path at t=0.
            w_sb = persist.tile([P, 3, HC, N_DC, Dh], bf16, tag="wqkv")
            nc.sync.dma_start(w_sb[:, 0, 0], wqkv_ap[:, 0, 0])
            xT0_sb = const.tile([P, N_DC, SQ], bf16, tag="xT0")
            for c in range(4):
                nc.sync.dma_start(xT0_sb[:, 4 * c:4 * (c + 1)],
                                  xT_ap[0, :, 4 * c:4 * (c + 1)])
            for h in range(HC):
                for part in range(3):
                    if (h, part) == (0, 0):
                        continue
                    nc.sync.dma_start(w_sb[:, part, h], wqkv_ap[:, part, h])

            ident = const.tile([P, P], f32, tag="ident")
            make_identity(nc, ident)
            ident16 = const.tile([P, P], f16, tag="ident16")
            nc.scalar.copy(ident16, ident)
            aux_sb = const.tile([P, 8 + HC * Dh], f32, tag="aux")
            nc.scalar.dma_start(aux_sb[:], aux.ap())
            masks_sb = const.tile([P, 4, 512], f16, tag="masks")
            nc.scalar.dma_start(masks_sb[:], masks_ap)
            bq_sb = aux_sb[:, 0:HC]
            bk_sb = aux_sb[:, HC:2 * HC]
            bvb_sb = aux_sb[:, 8:8 + HC * Dh]

            wo_sb = persist.tile([P, HC, D], f16, tag="wo")
            for h in range(HC):
                nc.sync.dma_start(wo_sb[:, h], wout_ap[:, h])

            kT_sb = persist.tile([P, HC, S], bf16, tag="kT")
            vaug_sb = persist.tile([P, HC, N_SC, VW], f16, tag="vaug")
            # ones column for softmax denominators
            nc.vector.memset(vaug_sb[:, :, :, Dh:Dh + 1], 1.0)

            with tc.tile_pool(name="abpool", bufs=2) as abpool, \
                 tc.tile_pool(name="exppool", bufs=CFG["EXPBUFS"]) as exppool, \
                 tc.tile_pool(name="spool", bufs=CFG["SPB"]) as spool, \
                 tc.tile_pool(name="qkps", bufs=CFG["QKPSB"], space="PSUM") as qkps, \
                 tc.tile_pool(name="trps", bufs=CFG["TRB"], space="PSUM") as trps, \
                 tc.tile_pool(name="qk2ps", bufs=CFG["QK2B"], space="PSUM") as qk2ps, \
                 tc.tile_pool(name="pvps", bufs=CFG["PVB"], space="PSUM") as pvps, \
                 tc.tile_pool(name="osb", bufs=CFG["OSB"]) as osb, \
                 tc.tile_pool(name="ops", bufs=CFG["OPSB"], space="PSUM") as ops:

                for sf in range(SPLIT):
                    # ---- Phase A: projections for this S-chunk ----
                    if sf == 0:
                        xT_sb = xT0_sb
                    else:
                        xT_sb = abpool.tile([P, N_DC, SQ], bf16, tag="xT")
                        nc.sync.dma_start(xT_sb[:], xT_ap[sf])
                    qT_sb = abpool.tile([P, HC, SQ], bf16, tag="qT")
                    vT_sb = abpool.tile([P, HC, SQ], f16, tag="vT")

                    for h in range(HC):
                        for part in range(3):
                            ps = qkps.tile([P, SQ], f32, tag="projps")
                            for dc in range(N_DC):
                                nc.tensor.matmul(
                                    ps,
                                    w_sb[:, part, h, dc],
                                    xT_sb[:, dc],
                                    start=(dc == 0), stop=(dc == N_DC - 1))
                            if part == 0:
                                nc.vector.tensor_scalar_add(
                                    qT_sb[:, h], ps, bq_sb[:, h:h + 1])
                            elif part == 1:
                                nc.vector.tensor_scalar_add(
                                    kT_sb[:, h, sf * SQ:(sf + 1) * SQ],
                                    ps, bk_sb[:, h:h + 1])
                            else:
                                nc.vector.tensor_copy(vT_sb[:, h], ps)

                    # v: transpose to natural layout, add bias, cast to fp16
                    for h in range(HC):
                        for scl in range(SQ // P):
                            sc = sf * (SQ // P) + scl
                            tp = trps.tile([P, P], f16, tag="tr")
                            nc.tensor.transpose(
                                tp, vT_sb[:, h, scl * P:(scl + 1) * P], ident16)
                            nc.vector.tensor_add(
                                vaug_sb[:, h, sc, 0:Dh], tp,
                                bvb_sb[:, h * Dh:(h + 1) * Dh])

                    # ---- Phase B: attention for q rows in this S-chunk ----
                    attnT_sb = abpool.tile([P, HC, SQ], f16, tag="attnT")
                    for h in range(HC):
                        q0 = sf * SQ
                        nk = (q0 + SQ) // P
                        exps = []
                        for kc in range(nk):
                            j = kc - (nk - 4)
                            # columns < j*128 of this chunk are fully masked
                            off = 0 if j < 1 else j * P
                            w = SQ - off
                            qk = qk2ps.tile([P, SQ], f32, tag="qk")
                            nc.tensor.matmul(
                                qk[:, 0:w],
                                kT_sb[:, h, kc * P:(kc + 1) * P],
                                qT_sb[:, h, off:SQ],
                                start=True, stop=True)
                            ex = exppool.tile([P, SQ], f16, tag="exp")
                            nc.scalar.activation(
                                ex[:, off:SQ], qk[:, 0:w],
                                mybir.ActivationFunctionType.Exp,
                                scale=SCALE)
                            if j >= 0:
                                nc.vector.tensor_mul(
                                    ex[:, off:SQ], ex[:, off:SQ],
                                    masks_sb[:, j, off:SQ])
                            exps.append(ex)
                        for sub in range(4):
                            nkq = sf * 4 + sub + 1
                            ps = pvps.tile([P, VW], f32, tag="pv")
                            for kc in range(nkq):
                                nc.tensor.matmul(
                                    ps[:, 0:Dh + 1],
                                    exps[kc][:, sub * P:(sub + 1) * P],
                                    vaug_sb[:, h, kc, 0:Dh + 1],
                                    start=(kc == 0),
                                    stop=(kc == nkq - 1))
                            rc = spool.tile([P, 1], f32, tag="rc")
                            nc.vector.reciprocal(rc, ps[:, Dh:Dh + 1])
                            at = spool.tile([P, P], f16, tag="at")
                            nc.vector.tensor_mul(
                                at, ps[:, 0:Dh], rc.to_broadcast((P, P)))
                            tp = trps.tile([P, P], f16, tag="tr")
                            nc.tensor.transpose(tp, at, ident16)
                            nc.vector.tensor_copy(
                                attnT_sb[:, h, sub * P:(sub + 1) * P], tp)

                    # ---- Phase C (this S-chunk): partial out projection ----
                    for ssl in range(SQ // P):
                        ss = sf * (SQ // P) + ssl
                        ot = osb.tile([P, D], f16, tag="ot")
                        for n in range(D // 512):
                            ps_o = ops.tile([P, 512], f32, tag="ops")
                            for hc in range(HC):
                                nc.tensor.matmul(
                                    ps_o,
                                    attnT_sb[:, hc, ssl * P:(ssl + 1) * P],
                                    wo_sb[:, hc, n * 512:(n + 1) * 512],
                                    start=(hc == 0),
                                    stop=(hc == HC - 1))
                            nc.vector.tensor_copy(ot[:, n * 512:(n + 1) * 512],
                                                  ps_o)
                        nc.sync.dma_start(outp_ap[ss * P:(ss + 1) * P], ot)

    nc.compile()
    return nc


def shard_inputs(x, Wqkv, bqkv, Wout):
    """Build the 8 per-core input maps."""
    mask = np.zeros((P, 4, 512), np.float16)
    kk = np.arange(P)[:, None]
    qq = np.arange(512)[None, :]
    for j in range(4):
        mask[:, j] = (qq >= kk + P * j).astype(np.float16)

    # xT packed per batch: [sf, p, dc, q] so each sf slice is one
    # contiguous-per-partition DMA
    xTs = [np.ascontiguousarray(
        x[b].T.reshape(N_DC, P, SPLIT, SQ).transpose(2, 1, 0, 3))
        .astype(np_bf16) for b in range(B)]
    # Wqkv [D, 3*H*Dh] -> [p, part, h, dc, dh]; per-head-group slices are
    # contiguous copies in bf16
    W_all = Wqkv.reshape(N_DC, P, 3, H, Dh).transpose(1, 2, 3, 0, 4)
    wout_f16 = Wout.astype(np.float16)
    per_hg = {}
    for hg in range(4):
        h0 = hg * HC
        c0 = h0 * Dh
        cw = HC * Dh
        aux = np.zeros((P, 8 + cw), np.float32)
        aux[:, 0:HC] = bqkv[c0:c0 + cw].reshape(HC, P).T
        aux[:, HC:2 * HC] = (bqkv[H * Dh + c0:H * Dh + c0 + cw]
                             .reshape(HC, P).T)
        aux[:, 8:8 + cw] = bqkv[2 * H * Dh + c0:2 * H * Dh + c0 + cw][None, :]
        per_hg[hg] = dict(
            wqkv=np.ascontiguousarray(W_all[:, :, h0:h0 + HC]).astype(np_bf16),
            wout=np.ascontiguousarray(
                wout_f16[c0:c0 + cw].reshape(HC, P, D).transpose(1, 0, 2)),
            aux=aux,
        )
    in_maps = []
    for c in range(8):
        b, hg = divmod(c, 4)
        g = per_hg[hg]
        in_maps.append({
            "xT": xTs[b], "wqkv": g["wqkv"], "wout": g["wout"],
            "aux": g["aux"], "masks": mask,
        })
    return in_maps


def kernel(x, Wqkv, bqkv, Wout, bout):
    x = np.asarray(x, dtype=np.float32)
    Wqkv = np.asarray(Wqkv, dtype=np.float32)
    bqkv = np.asarray(bqkv, dtype=np.float32)
    Wout = np.asarray(Wout, dtype=np.float32)
    bout = np.asarray(bout, dtype=np.float32)

    if "nc" not in _COMPILED:
        _COMPILED["nc"] = build_program()
    nc = _COMPILED["nc"]

    in_maps = shard_inputs(x, Wqkv, bqkv, Wout)
    res = run_bass_kernel_spmd(nc, in_maps, core_ids=list(range(8)))

    out = np.empty((B, S, D), np.float32)
    for b in range(B):
        np.copyto(out[b], res.results[4 * b]["outp"].astype(np.float32))
        for c in range(4 * b + 1, 4 * b + 4):
            np.add(out[b], res.results[c]["outp"].astype(np.float32),
                   out=out[b])
        out[b] += bout[None, :]
    return out
